# revision 7
# baseline (speedup 1.0000x reference)
"""Trainium2 Bass kernel for a 2-layer GAT (nn_GAT_44839458571021).

Strategy (8 NeuronCores, SPMD, one NEFF):
  * conv1 linear (x @ W1aug) is computed replicated on every core into a
    DRAM gather table h1aug[50000, 448] whose rows interleave per-head
    outputs with a 1.0 "ones slot" (so the segment matmul produces softmax
    numerators and denominators together), plus a_src/a_dst columns.
  * Edges (with self loops) are sorted by destination on the host and
    sharded by destination range: core k owns dst nodes [6250k, 6250(k+1)).
    Per 128-dst window, per-edge rows are fetched with dma_gather (int16
    indices -> the table is addressed as two halves split at 32768), edge
    softmax weights are computed on-chip (a_dst expanded per-edge via a
    one-hot transpose matmul), and the segmented sum over each window is a
    one-hot matmul accumulated in PSUM.
  * conv2 rows (relu(out1+b1) @ W2aug plus ones/a_src2/a_dst2 slots) are
    computed per window into a local shard table, AllGathered to a full
    [50000, 128] table, and conv2 aggregation runs the same way.
  * Global mean pooling: per-window one-hot matmul accumulates per-graph
    partial sums; the host sums partials across cores and divides by the
    graph sizes.

Host-side work is limited to index bookkeeping (sort/group/pad of edge
indices) and weight augmentation; all FLOPs over node/edge data run on
device.
"""

import math

import numpy as np

import concourse.bacc as bacc
import concourse.mybir as mybir
import concourse.tile as tile
from concourse.bass_utils import run_bass_kernel_spmd
from concourse.masks import make_identity

# ---- geometry (hardcoded for this problem) ----
N = 50000
E = 800000
G = 256
F_IN = 128
H1, D1 = 12, 32
H2, D2 = 1, 64
NEG = 0.2
NC = 8
P = 128
SH = N // NC                    # 6250 dst nodes per core
NWIN = (SH + P - 1) // P        # 49 windows per core
HALF = 32768                    # int16 gather index limit -> split tables
C1_COLS = 448                   # conv1 table row, f32 (1792B, %256)
C2_COLS = 128                   # conv2 table row, f32 (512B, %256)
GC_PAD = 4                      # pad for graph-count shape stability
GROUP_C = 4                     # chunks (128 edges each) per op group

f32 = mybir.dt.float32
i16 = mybir.dt.int16


# ---------------------------------------------------------------- host prep

def _build_weights(W1, att_src1, att_dst1, W2, att_src2, att_dst2):
    W1 = np.ascontiguousarray(np.asarray(W1, np.float32))
    W2 = np.ascontiguousarray(np.asarray(W2, np.float32))
    att_src1 = np.asarray(att_src1, np.float32)
    att_dst1 = np.asarray(att_dst1, np.float32)
    att_src2 = np.asarray(att_src2, np.float32).reshape(-1)
    att_dst2 = np.asarray(att_dst2, np.float32).reshape(-1)

    W1aug = np.zeros((F_IN, C1_COLS), np.float32)
    for h in range(H1):
        W1aug[:, 33 * h:33 * h + 32] = W1[:, 32 * h:32 * h + 32]
        W1aug[:, 396 + h] = W1[:, 32 * h:32 * h + 32] @ att_src1[h]
        W1aug[:, 408 + h] = W1[:, 32 * h:32 * h + 32] @ att_dst1[h]

    W2aug = np.zeros((H1 * D1, 67), np.float32)
    W2aug[:, :64] = W2
    W2aug[:, 65] = W2 @ att_src2
    W2aug[:, 66] = W2 @ att_dst2
    # pad rows to 3*128 = 384 exactly (already 384)
    return W1aug, W2aug


def _build_edges(edge_index, batch):
    src = np.concatenate([np.asarray(edge_index[0], np.int64),
                          np.arange(N, dtype=np.int64)])
    dst = np.concatenate([np.asarray(edge_index[1], np.int64),
                          np.arange(N, dtype=np.int64)])
    order = np.argsort(dst, kind="stable")
    src, dst = src[order], dst[order]

    per = [[None] * NWIN for _ in range(NC)]
    for k in range(NC):
        base = k * SH
        for w in range(NWIN):
            lo = base + w * P
            hi = min(lo + P, base + SH)
            e0 = np.searchsorted(dst, lo, "left")
            e1 = np.searchsorted(dst, hi, "left")
            s = src[e0:e1]
            d = dst[e0:e1]
            selA = s < HALF
            per[k][w] = (s[selA], d[selA] - lo, s[~selA] - HALF, d[~selA] - lo)

    sched = []
    for w in range(NWIN):
        nA = max(len(per[k][w][0]) for k in range(NC))
        nB = max(len(per[k][w][2]) for k in range(NC))
        sched.append(dict(nd=min(P, SH - w * P),
                          chunksA=(nA + P - 1) // P,
                          chunksB=(nB + P - 1) // P))
    totch = sum(s["chunksA"] + s["chunksB"] for s in sched)

    idx16 = np.zeros((NC, 128, 8 * totch), np.int16)
    dstloc = np.full((NC, 128, totch), 999.0, np.float32)
    for k in range(NC):
        c0 = 0
        for w in range(NWIN):
            for (srcs, dls, nch) in (
                (per[k][w][0], per[k][w][1], sched[w]["chunksA"]),
                (per[k][w][2], per[k][w][3], sched[w]["chunksB"]),
            ):
                if nch == 0:
                    continue
                npad = nch * P
                idx = np.zeros(npad, np.int16)
                idx[:len(srcs)] = srcs
                dl = np.full(npad, 999.0, np.float32)
                dl[:len(dls)] = dls
                wr = idx.reshape(-1, 16).T
                idx16[k, :, 8 * c0: 8 * c0 + npad // 16] = np.tile(wr, (8, 1))
                dstloc[k, :, c0:c0 + nch] = dl.reshape(nch, P).T
                c0 += nch
        assert c0 == totch

    batch = np.asarray(batch, np.int64)
    g_lo = np.array([int(batch[k * SH]) for k in range(NC)])
    n_graphs = np.array([int(batch[(k + 1) * SH - 1]) - g_lo[k] + 1
                         for k in range(NC)])
    GC = int(-(-int(n_graphs.max()) // GC_PAD) * GC_PAD)
    assert GC <= P, "graph shard spans more than 128 graphs"
    batchloc = np.full((NC, 128, NWIN), 999.0, np.float32)
    for k in range(NC):
        bl = (batch[k * SH:(k + 1) * SH] - g_lo[k]).astype(np.float32)
        pad = np.full(NWIN * P - SH, 999.0, np.float32)
        batchloc[k] = np.concatenate([bl, pad]).reshape(NWIN, P).T

    return dict(idx16=idx16, dstloc=dstloc, sched=sched, totch=totch,
                g_lo=g_lo, n_graphs=n_graphs, GC=GC, batchloc=batchloc)


# ------------------------------------------------------------- device build

def _build_program(sched, totch, GC):
    nc = bacc.Bacc(None, target_bir_lowering=False, name="gat8")

    x_in = nc.dram_tensor("x", [N, F_IN], f32, kind="ExternalInput")
    xsh_in = nc.dram_tensor("xsh", [SH, F_IN], f32, kind="ExternalInput")
    w1_in = nc.dram_tensor("w1aug", [F_IN, C1_COLS], f32, kind="ExternalInput")
    w2_in = nc.dram_tensor("w2aug", [3 * P, 67], f32, kind="ExternalInput")
    b1_in = nc.dram_tensor("b1", [1, 384], f32, kind="ExternalInput")
    b2_in = nc.dram_tensor("b2", [1, 64], f32, kind="ExternalInput")
    idx_in = nc.dram_tensor("idx16", [128, 8 * totch], i16, kind="ExternalInput")
    dl_in = nc.dram_tensor("dstloc", [128, totch], f32, kind="ExternalInput")
    bl_in = nc.dram_tensor("batchloc", [128, NWIN], f32, kind="ExternalInput")
    pool_out = nc.dram_tensor("pool_out", [GC, 64], f32, kind="ExternalOutput")

    ALU = mybir.AluOpType
    ACT = mybir.ActivationFunctionType
    NT1 = math.ceil(N / P)

    with tile.TileContext(nc) as tc:
        with (
            tc.tile_pool(name="const", bufs=1) as cp,
            tc.tile_pool(name="dram", bufs=1, space="DRAM") as dp,
            tc.tile_pool(name="work", bufs=2) as wp,
            tc.tile_pool(name="gath", bufs=3) as gp,
            tc.tile_pool(name="ps_acc", bufs=2, space="PSUM") as pacc,
            tc.tile_pool(name="ps_pt", bufs=2, space="PSUM") as ppt,
            tc.tile_pool(name="ps_ad", bufs=1, space="PSUM") as pad,
            tc.tile_pool(name="ps_sm", bufs=2, space="PSUM") as psm,
            tc.tile_pool(name="ps_pool", bufs=1, space="PSUM") as ppool,
        ):
            tab1 = dp.tile([N, C1_COLS], f32, tag="tab1")
            shard2 = dp.tile([SH, C2_COLS], f32, tag="shard2")
            tab2 = dp.tile([N, C2_COLS], f32, tag="tab2", addr_space="Shared")

            ident = cp.tile([P, P], f32, tag="ident")
            make_identity(nc, ident[:])
            iota = cp.tile([P, P], f32, tag="iota")
            nc.gpsimd.iota(iota[:], pattern=[[1, P]], base=0,
                           channel_multiplier=0,
                           allow_small_or_imprecise_dtypes=True)
            w1t = cp.tile([F_IN, C1_COLS], f32, tag="w1t")
            nc.sync.dma_start(w1t[:], w1_in[:])
            w2t = cp.tile([P, 3, 67], f32, tag="w2t")
            for c in range(3):
                nc.sync.dma_start(w2t[:, c, :], w2_in[c * P:(c + 1) * P, :])
            b1r = cp.tile([1, 384], f32, tag="b1r")
            nc.sync.dma_start(b1r[:], b1_in[:])
            b1t = cp.tile([P, 384], f32, tag="b1t")
            nc.gpsimd.partition_broadcast(b1t[:], b1r[:])
            b2r = cp.tile([1, 64], f32, tag="b2r")
            nc.sync.dma_start(b2r[:], b2_in[:])
            b2t = cp.tile([P, 64], f32, tag="b2t")
            nc.gpsimd.partition_broadcast(b2t[:], b2r[:])
            idxt = cp.tile([128, 8 * totch], i16, tag="idxt")
            nc.sync.dma_start(idxt[:], idx_in[:])
            dlt = cp.tile([128, totch], f32, tag="dlt")
            nc.sync.dma_start(dlt[:], dl_in[:])
            blt = cp.tile([128, NWIN], f32, tag="blt")
            nc.sync.dma_start(blt[:], bl_in[:])
            stash = cp.tile([P, NWIN], f32, tag="stash")

            # ---------------- phase L: h1aug table (replicated) ----------
            for t in range(NT1):
                r0 = t * P
                nr = min(P, N - r0)
                xt = wp.tile([P, F_IN], f32, tag="xt")
                nc.sync.dma_start(xt[:nr], x_in[r0:r0 + nr, :])
                xT_ps = psm.tile([P, P], f32, tag="wps")
                nc.tensor.transpose(xT_ps[:], xt[:], ident[:])
                xT = wp.tile([P, P], f32, tag="xT")
                nc.vector.tensor_copy(xT[:], xT_ps[:])
                h_ps = pacc.tile([P, C1_COLS], f32, tag="acc")
                nc.tensor.matmul(h_ps[:], lhsT=xT[:], rhs=w1t[:],
                                 start=True, stop=True)
                ht = wp.tile([P, C1_COLS], f32, tag="ht")
                nc.scalar.copy(ht[:], h_ps[:])
                ones_view = ht[:, 0:396].rearrange("p (h t) -> p h t", t=33)[:, :, 32]
                nc.vector.memset(ones_view, 1.0)
                nc.sync.dma_start(tab1[r0:r0 + nr, :], ht[:nr])

            # ---------------- phase A1: conv1 aggregation ----------------
            def onehot_group(c0, gn, tag_prefix):
                """Build P4 [P, gn, P] one-hot and Pt (transposed) in SBUF."""
                P4 = wp.tile([P, GROUP_C, P], f32, tag=tag_prefix + "P4")
                nc.vector.tensor_tensor(
                    out=P4[:, :gn, :],
                    in0=iota[:].unsqueeze(1).to_broadcast([P, gn, P]),
                    in1=dlt[:, c0:c0 + gn].unsqueeze(2).to_broadcast([P, gn, P]),
                    op=ALU.is_equal,
                )
                Pt_ps = ppt.tile([P, GROUP_C, P], f32, tag="Ptp")
                for c in range(gn):
                    nc.tensor.transpose(Pt_ps[:, c], P4[:, c, :], ident[:])
                Pt = wp.tile([P, GROUP_C, P], f32, tag=tag_prefix + "Pt")
                nc.vector.tensor_copy(Pt[:, :gn], Pt_ps[:, :gn])
                return P4, Pt

            c0 = 0
            for w in range(NWIN):
                s = sched[w]
                nd = s["nd"]
                w0 = w * P
                # a_dst for this window's dst rows, from x shard
                xw = wp.tile([P, F_IN], f32, tag="xw")
                nc.sync.dma_start(xw[:nd], xsh_in[w0:w0 + nd, :])
                xwT_ps = psm.tile([P, P], f32, tag="wps")
                nc.tensor.transpose(xwT_ps[:], xw[:], ident[:])
                xwT = wp.tile([P, P], f32, tag="xwT")
                nc.vector.tensor_copy(xwT[:], xwT_ps[:])
                ad_ps = psm.tile([P, H1], f32, tag="wps")
                nc.tensor.matmul(ad_ps[:], lhsT=xwT[:], rhs=w1t[:, 408:420],
                                 start=True, stop=True)
                adw = wp.tile([P, H1], f32, tag="adw")
                nc.vector.tensor_copy(adw[:], ad_ps[:])

                ps_full = pacc.tile([P, C1_COLS], f32, tag="acc", name="ps_full")
                ps_out = ps_full[:, 0:396]
                nch_w = s["chunksA"] + s["chunksB"]
                ci = 0
                for half, nch in (("A", s["chunksA"]), ("B", s["chunksB"])):
                    tab_ap = tab1[0:HALF, :] if half == "A" else tab1[HALF:N, :]
                    for g0 in range(0, nch, GROUP_C):
                        gn = min(GROUP_C, nch - g0)
                        gc = c0 + ci
                        V = gp.tile([P, GROUP_C, C1_COLS], f32, tag="V1")
                        nc.gpsimd.dma_gather(
                            out_ap=V[:, :gn, :], in_ap=tab_ap,
                            idxs_ap=idxt[:, 8 * gc: 8 * (gc + gn)],
                            num_idxs=gn * P, num_idxs_reg=gn * P,
                            elem_size=C1_COLS,
                        )
                        P4, Pt = onehot_group(gc, gn, "a1")
                        adst_ps = pad.tile([P, GROUP_C, H1], f32, tag="adp")
                        for c in range(gn):
                            nc.tensor.matmul(adst_ps[:, c], lhsT=Pt[:, c, :],
                                             rhs=adw[:], start=True, stop=True)
                        wv = wp.tile([P, GROUP_C, H1], f32, tag="wv1")
                        nc.vector.tensor_tensor(
                            out=wv[:, :gn], in0=V[:, :gn, 396:408],
                            in1=adst_ps[:, :gn], op=ALU.add)
                        wl = wp.tile([P, GROUP_C, H1], f32, tag="wl1")
                        nc.vector.tensor_scalar_mul(wl[:, :gn], wv[:, :gn], NEG)
                        nc.vector.tensor_tensor(out=wl[:, :gn], in0=wl[:, :gn],
                                                in1=wv[:, :gn], op=ALU.max)
                        nc.scalar.activation(wv[:, :gn], wl[:, :gn], ACT.Exp)
                        nc.vector.tensor_tensor(
                            out=V[:, :gn, 0:396].rearrange(
                                "p c (h t) -> p c h t", t=33),
                            in0=V[:, :gn, 0:396].rearrange(
                                "p c (h t) -> p c h t", t=33),
                            in1=wv[:, :gn].unsqueeze(3).to_broadcast(
                                [P, gn, H1, 33]),
                            op=ALU.mult,
                        )
                        for c in range(gn):
                            nc.tensor.matmul(
                                ps_out[:], lhsT=P4[:, c, :], rhs=V[:, c, 0:396],
                                start=(ci + c == 0), stop=(ci + c == nch_w - 1),
                            )
                        ci += gn
                c0 += nch_w

                # epilogue: normalize, relu(+bias), conv2 rows
                # (clamp denominator: padded dst rows accumulate exactly 0;
                #  max with a tiny value keeps 0 * recip finite)
                rec = wp.tile([P, H1], f32, tag="rec")
                nc.vector.tensor_scalar_max(
                    rec[:], ps_out[:, 0:396].rearrange(
                        "p (h t) -> p h t", t=33)[:, :, 32], 1e-30)
                nc.vector.reciprocal(rec[:], rec[:])
                out1 = wp.tile([P, 384], f32, tag="out1")
                nc.vector.tensor_tensor(
                    out=out1[:].rearrange("p (h t) -> p h t", t=32),
                    in0=ps_out[:, 0:396].rearrange(
                        "p (h t) -> p h t", t=33)[:, :, 0:32],
                    in1=rec[:].unsqueeze(2).to_broadcast([P, H1, 32]),
                    op=ALU.mult,
                )
                nc.vector.tensor_tensor(out=out1[:], in0=out1[:], in1=b1t[:],
                                        op=ALU.add)
                nc.vector.tensor_scalar_max(out1[:], out1[:], 0.0)
                o1T_ps = psm.tile([P, 3, P], f32, tag="wps")
                for c in range(3):
                    nc.tensor.transpose(o1T_ps[:, c],
                                        out1[:, c * P:(c + 1) * P], ident[:])
                o1T = wp.tile([P, 3, P], f32, tag="o1T")
                nc.vector.tensor_copy(o1T[:], o1T_ps[:])
                h2_ps = psm.tile([P, 67], f32, tag="wps")
                for c in range(3):
                    nc.tensor.matmul(h2_ps[:], lhsT=o1T[:, c, :],
                                     rhs=w2t[:, c, :],
                                     start=(c == 0), stop=(c == 2))
                nc.vector.tensor_copy(stash[:, w:w + 1], h2_ps[:, 66:67])
                h2t = wp.tile([P, C2_COLS], f32, tag="h2t")
                nc.scalar.copy(h2t[:, 0:67], h2_ps[:])
                nc.vector.memset(h2t[:, 64:65], 1.0)
                nc.vector.memset(h2t[:, 67:], 0.0)
                nc.sync.dma_start(shard2[w0:w0 + nd, :], h2t[:nd])

            # ---------------- allgather conv2 table ----------------------
            nc.gpsimd.collective_compute(
                "AllGather", mybir.AluOpType.bypass,
                replica_groups=[list(range(NC))],
                ins=[shard2[:].opt()],
                outs=[tab2[:].opt()],
            )

            # ---------------- phase A2: conv2 aggregation + pooling ------
            pool_ps = ppool.tile([GC, 64], f32, tag="pool_ps")
            c0 = 0
            for w in range(NWIN):
                s = sched[w]
                nd = s["nd"]
                ps2_full = pacc.tile([P, C1_COLS], f32, tag="acc", name="ps2_full")
                ps2 = ps2_full[:, 0:65]
                nch_w = s["chunksA"] + s["chunksB"]
                ci = 0
                for half, nch in (("A", s["chunksA"]), ("B", s["chunksB"])):
                    tab_ap = tab2[0:HALF, :] if half == "A" else tab2[HALF:N, :]
                    for g0 in range(0, nch, GROUP_C):
                        gn = min(GROUP_C, nch - g0)
                        gc = c0 + ci
                        V2 = gp.tile([P, GROUP_C, C2_COLS], f32, tag="V2")
                        nc.gpsimd.dma_gather(
                            out_ap=V2[:, :gn, :], in_ap=tab_ap,
                            idxs_ap=idxt[:, 8 * gc: 8 * (gc + gn)],
                            num_idxs=gn * P, num_idxs_reg=gn * P,
                            elem_size=C2_COLS,
                        )
                        P4, Pt = onehot_group(gc, gn, "a2")
                        adst_ps_full = pad.tile([P, GROUP_C, H1], f32, tag="adp", name="adst_ps_full")
                        adst_ps = adst_ps_full[:, :, 0:1]
                        for c in range(gn):
                            nc.tensor.matmul(adst_ps[:, c], lhsT=Pt[:, c, :],
                                             rhs=stash[:, w:w + 1],
                                             start=True, stop=True)
                        wv = wp.tile([P, GROUP_C], f32, tag="wv2")
                        nc.vector.tensor_tensor(
                            out=wv[:, :gn], in0=V2[:, :gn, 65],
                            in1=adst_ps[:, :gn, 0], op=ALU.add)
                        wl = wp.tile([P, GROUP_C], f32, tag="wl2")
                        nc.vector.tensor_scalar_mul(wl[:, :gn], wv[:, :gn], NEG)
                        nc.vector.tensor_tensor(out=wl[:, :gn], in0=wl[:, :gn],
                                                in1=wv[:, :gn], op=ALU.max)
                        nc.scalar.activation(wv[:, :gn], wl[:, :gn], ACT.Exp)
                        nc.vector.tensor_tensor(
                            out=V2[:, :gn, 0:65],
                            in0=V2[:, :gn, 0:65],
                            in1=wv[:, :gn].unsqueeze(2).to_broadcast(
                                [P, gn, 65]),
                            op=ALU.mult,
                        )
                        for c in range(gn):
                            nc.tensor.matmul(
                                ps2[:], lhsT=P4[:, c, :], rhs=V2[:, c, 0:65],
                                start=(ci + c == 0), stop=(ci + c == nch_w - 1),
                            )
                        ci += gn
                c0 += nch_w

                rec2 = wp.tile([P, 1], f32, tag="rec2")
                nc.vector.tensor_scalar_max(rec2[:], ps2[:, 64:65], 1e-30)
                nc.vector.reciprocal(rec2[:], rec2[:])
                out2 = wp.tile([P, 64], f32, tag="out2")
                nc.vector.tensor_scalar(out=out2[:], in0=ps2[:, 0:64],
                                        scalar1=rec2[:, 0:1], scalar2=None,
                                        op0=ALU.mult)
                nc.vector.tensor_tensor(out=out2[:], in0=out2[:], in1=b2t[:],
                                        op=ALU.add)
                Pg = wp.tile([P, GC], f32, tag="Pg")
                nc.vector.tensor_tensor(
                    out=Pg[:], in0=iota[:, 0:GC],
                    in1=blt[:, w:w + 1].to_broadcast([P, GC]),
                    op=ALU.is_equal)
                nc.tensor.matmul(pool_ps[:], lhsT=Pg[:nd, :], rhs=out2[:nd, :],
                                 start=(w == 0), stop=(w == NWIN - 1))

            pool_sb = cp.tile([GC, 64], f32, tag="pool_sb")
            nc.vector.tensor_copy(pool_sb[:], pool_ps[:])
            nc.sync.dma_start(pool_out[:], pool_sb[:])

    nc.compile()
    return nc


# ------------------------------------------------------------------ driver

_CACHE = {}


def _run(inputs, trace=False):
    x = np.ascontiguousarray(np.asarray(inputs["x"], np.float32))
    ed = _build_edges(inputs["edge_index"], inputs["batch"])
    W1aug, W2aug = _build_weights(
        inputs["W1"], inputs["att_src1"], inputs["att_dst1"],
        inputs["W2"], inputs["att_src2"], inputs["att_dst2"])
    b1 = np.asarray(inputs["bias1"], np.float32).reshape(1, 384)
    b2 = np.asarray(inputs["bias2"], np.float32).reshape(1, 64)

    key = (ed["totch"], ed["GC"],
           tuple((s["nd"], s["chunksA"], s["chunksB"]) for s in ed["sched"]))
    if key not in _CACHE:
        _CACHE.clear()
        _CACHE[key] = _build_program(ed["sched"], ed["totch"], ed["GC"])
    nc = _CACHE[key]

    in_maps = []
    for k in range(NC):
        in_maps.append(dict(
            x=x,
            xsh=np.ascontiguousarray(x[k * SH:(k + 1) * SH]),
            w1aug=W1aug, w2aug=np.ascontiguousarray(W2aug),
            b1=b1, b2=b2,
            idx16=np.ascontiguousarray(ed["idx16"][k]),
            dstloc=np.ascontiguousarray(ed["dstloc"][k]),
            batchloc=np.ascontiguousarray(ed["batchloc"][k]),
        ))
    res = run_bass_kernel_spmd(nc, in_maps, core_ids=list(range(NC)),
                               trace=trace)

    sums = np.zeros((G, 64), np.float64)
    GCn = ed["GC"]
    for k in range(NC):
        lo = int(ed["g_lo"][k])
        hi = min(lo + GCn, G)
        sums[lo:hi] += res.results[k]["pool_out"][:hi - lo]
    cnts = np.bincount(np.asarray(inputs["batch"], np.int64),
                       minlength=G).astype(np.float64)
    out = (sums / np.maximum(cnts, 1.0)[:, None]).astype(np.float32)
    return out, res


def kernel(**inputs) -> np.ndarray:
    out, _ = _run(inputs, trace=False)
    return out


# revision 9
# speedup vs baseline: 1.1470x; 1.1470x over previous
"""Trainium2 Bass kernel for a 2-layer GAT (nn_GAT_44839458571021).

Strategy (8 NeuronCores, SPMD, one NEFF):
  * conv1 linear (x @ W1aug) is computed replicated on every core into a
    DRAM gather table h1aug[50000, 448] whose rows interleave per-head
    outputs with a 1.0 "ones slot" (so the segment matmul produces softmax
    numerators and denominators together), plus a_src/a_dst columns.
  * Edges (with self loops) are sorted by destination on the host and
    sharded by destination range: core k owns dst nodes [6250k, 6250(k+1)).
    Per 128-dst window, per-edge rows are fetched with dma_gather (int16
    indices -> the table is addressed as two halves split at 32768), edge
    softmax weights are computed on-chip (a_dst expanded per-edge via a
    one-hot transpose matmul), and the segmented sum over each window is a
    one-hot matmul accumulated in PSUM.
  * conv2 rows (relu(out1+b1) @ W2aug plus ones/a_src2/a_dst2 slots) are
    computed per window into a local shard table, AllGathered to a full
    [50000, 128] table, and conv2 aggregation runs the same way.
  * Global mean pooling: per-window one-hot matmul accumulates per-graph
    partial sums; the host sums partials across cores and divides by the
    graph sizes.

Matmul precision: the PE pays 4 cycles/row + slow weight loads for fp32,
so all bulk matmuls run as bf16 PAIRS (hi + lo = exact split of the fp32
value; one-hot / identity stationary operands are exactly representable in
bf16), accumulated in fp32 PSUM.  This keeps end-to-end error near fp32
while running the PE at bf16 rates.

Host-side work is limited to index bookkeeping (sort/group/pad of edge
indices), weight augmentation, and memory-layout transposes of inputs;
all FLOPs over node/edge data run on device.
"""

import math

import numpy as np

import concourse.bacc as bacc
import concourse.mybir as mybir
import concourse.tile as tile
from concourse.bass_utils import run_bass_kernel_spmd
from concourse.masks import make_identity

# ---- geometry (hardcoded for this problem) ----
N = 50000
E = 800000
G = 256
F_IN = 128
H1, D1 = 12, 32
H2, D2 = 1, 64
NEG = 0.2
NC = 8
P = 128
SH = N // NC                    # 6250 dst nodes per core
NWIN = (SH + P - 1) // P        # 49 windows per core
HALF = 32768                    # int16 gather index limit -> split tables
C1_COLS = 448                   # conv1 table row, f32 (1792B, %256)
C2_COLS = 128                   # conv2 table row, f32 (512B, %256)
GC_PAD = 4                      # pad for graph-count shape stability
GROUP_C = 4                     # chunks (128 edges each) per op group
GCALL = 4                       # chunks per dma_gather call

f32 = mybir.dt.float32
bf16 = mybir.dt.bfloat16
i16 = mybir.dt.int16


# ---------------------------------------------------------------- host prep

def _bf_split(a):
    hi = a.astype(np.float32).astype(mybir.dt.np(bf16))
    lo = (a - hi.astype(np.float32)).astype(mybir.dt.np(bf16))
    return hi, lo


def _build_weights(W1, att_src1, att_dst1, W2, att_src2, att_dst2):
    W1 = np.ascontiguousarray(np.asarray(W1, np.float32))
    W2 = np.ascontiguousarray(np.asarray(W2, np.float32))
    att_src1 = np.asarray(att_src1, np.float32)
    att_dst1 = np.asarray(att_dst1, np.float32)
    att_src2 = np.asarray(att_src2, np.float32).reshape(-1)
    att_dst2 = np.asarray(att_dst2, np.float32).reshape(-1)

    W1aug = np.zeros((F_IN, C1_COLS), np.float32)
    for h in range(H1):
        W1aug[:, 33 * h:33 * h + 32] = W1[:, 32 * h:32 * h + 32]
        W1aug[:, 396 + h] = W1[:, 32 * h:32 * h + 32] @ att_src1[h]
        W1aug[:, 408 + h] = W1[:, 32 * h:32 * h + 32] @ att_dst1[h]

    W2aug = np.zeros((H1 * D1, 67), np.float32)
    W2aug[:, :64] = W2
    W2aug[:, 65] = W2 @ att_src2
    W2aug[:, 66] = W2 @ att_dst2
    return W1aug, W2aug


def _build_edges(edge_index, batch):
    src = np.concatenate([np.asarray(edge_index[0], np.int64),
                          np.arange(N, dtype=np.int64)])
    dst = np.concatenate([np.asarray(edge_index[1], np.int64),
                          np.arange(N, dtype=np.int64)])
    order = np.argsort(dst, kind="stable")
    src, dst = src[order], dst[order]

    per = [[None] * NWIN for _ in range(NC)]
    for k in range(NC):
        base = k * SH
        for w in range(NWIN):
            lo = base + w * P
            hi = min(lo + P, base + SH)
            e0 = np.searchsorted(dst, lo, "left")
            e1 = np.searchsorted(dst, hi, "left")
            s = src[e0:e1]
            d = dst[e0:e1]
            selA = s < HALF
            per[k][w] = (s[selA], d[selA] - lo, s[~selA] - HALF, d[~selA] - lo)

    sched = []
    for w in range(NWIN):
        nA = max(len(per[k][w][0]) for k in range(NC))
        nB = max(len(per[k][w][2]) for k in range(NC))
        sched.append(dict(nd=min(P, SH - w * P),
                          chunksA=(nA + P - 1) // P,
                          chunksB=(nB + P - 1) // P))
    totch = sum(s["chunksA"] + s["chunksB"] for s in sched)

    idx16 = np.zeros((NC, 128, 8 * totch), np.int16)
    dstloc = np.full((NC, 128, totch), 999.0, np.float32)
    for k in range(NC):
        c0 = 0
        for w in range(NWIN):
            for (srcs, dls, nch) in (
                (per[k][w][0], per[k][w][1], sched[w]["chunksA"]),
                (per[k][w][2], per[k][w][3], sched[w]["chunksB"]),
            ):
                if nch == 0:
                    continue
                npad = nch * P
                idx = np.zeros(npad, np.int16)
                idx[:len(srcs)] = srcs
                dl = np.full(npad, 999.0, np.float32)
                dl[:len(dls)] = dls
                wr = idx.reshape(-1, 16).T
                idx16[k, :, 8 * c0: 8 * c0 + npad // 16] = np.tile(wr, (8, 1))
                dstloc[k, :, c0:c0 + nch] = dl.reshape(nch, P).T
                c0 += nch
        assert c0 == totch

    batch = np.asarray(batch, np.int64)
    g_lo = np.array([int(batch[k * SH]) for k in range(NC)])
    n_graphs = np.array([int(batch[(k + 1) * SH - 1]) - g_lo[k] + 1
                         for k in range(NC)])
    GC = int(-(-int(n_graphs.max()) // GC_PAD) * GC_PAD)
    assert GC <= P, "graph shard spans more than 128 graphs"
    batchloc = np.full((NC, 128, NWIN), 999.0, np.float32)
    for k in range(NC):
        bl = (batch[k * SH:(k + 1) * SH] - g_lo[k]).astype(np.float32)
        pad = np.full(NWIN * P - SH, 999.0, np.float32)
        batchloc[k] = np.concatenate([bl, pad]).reshape(NWIN, P).T

    return dict(idx16=idx16, dstloc=dstloc, sched=sched, totch=totch,
                g_lo=g_lo, n_graphs=n_graphs, GC=GC, batchloc=batchloc)


# ------------------------------------------------------------- device build

def _build_program(sched, totch, GC):
    nc = bacc.Bacc(None, target_bir_lowering=False, name="gat8")

    CH1 = max(max(s["chunksA"], s["chunksB"]) for s in sched)

    xT_in = nc.dram_tensor("xT", [F_IN, N], f32, kind="ExternalInput")
    xshT_in = nc.dram_tensor("xshT", [F_IN, SH], f32, kind="ExternalInput")
    w1h_in = nc.dram_tensor("w1h", [F_IN, C1_COLS], bf16, kind="ExternalInput")
    w1l_in = nc.dram_tensor("w1l", [F_IN, C1_COLS], bf16, kind="ExternalInput")
    w1d_in = nc.dram_tensor("w1d", [F_IN, H1], f32, kind="ExternalInput")
    w2_in = nc.dram_tensor("w2aug", [3 * P, 67], f32, kind="ExternalInput")
    b1_in = nc.dram_tensor("b1", [1, 384], f32, kind="ExternalInput")
    b2_in = nc.dram_tensor("b2", [1, 64], f32, kind="ExternalInput")
    idx_in = nc.dram_tensor("idx16", [128, 8 * totch], i16, kind="ExternalInput")
    dl_in = nc.dram_tensor("dstloc", [128, totch], f32, kind="ExternalInput")
    bl_in = nc.dram_tensor("batchloc", [128, NWIN], f32, kind="ExternalInput")
    pool_out = nc.dram_tensor("pool_out", [GC, 64], f32, kind="ExternalOutput")

    ALU = mybir.AluOpType
    ACTF = mybir.ActivationFunctionType
    NT1 = math.ceil(N / P)

    with tile.TileContext(nc) as tc:
        with (
            tc.tile_pool(name="const", bufs=1) as cp,
            tc.tile_pool(name="dram", bufs=1, space="DRAM") as dp,
            tc.tile_pool(name="work", bufs=2) as wp,
            tc.tile_pool(name="gath", bufs=2) as gp,
            tc.tile_pool(name="ps_acc", bufs=2, space="PSUM") as pacc,
            tc.tile_pool(name="ps_pt", bufs=2, space="PSUM") as ppt,
            tc.tile_pool(name="ps_ad", bufs=1, space="PSUM") as pad,
            tc.tile_pool(name="ps_sm", bufs=2, space="PSUM") as psm,
            tc.tile_pool(name="ps_pool", bufs=1, space="PSUM") as ppool,
        ):
            tab1 = dp.tile([N, C1_COLS], f32, tag="tab1")
            shard2 = dp.tile([SH, C2_COLS], f32, tag="shard2")
            tab2 = dp.tile([N, C2_COLS], f32, tag="tab2", addr_space="Shared")

            identb = cp.tile([P, P], bf16, tag="identb")
            make_identity(nc, identb[:])
            ident = cp.tile([P, P], f32, tag="ident")
            make_identity(nc, ident[:])
            iota = cp.tile([P, P], f32, tag="iota")
            nc.gpsimd.iota(iota[:], pattern=[[1, P]], base=0,
                           channel_multiplier=0,
                           allow_small_or_imprecise_dtypes=True)
            w1h = cp.tile([F_IN, C1_COLS], bf16, tag="w1h")
            nc.sync.dma_start(w1h[:], w1h_in[:])
            w1l = cp.tile([F_IN, C1_COLS], bf16, tag="w1l")
            nc.sync.dma_start(w1l[:], w1l_in[:])
            w1d = cp.tile([F_IN, H1], f32, tag="w1d")
            nc.sync.dma_start(w1d[:], w1d_in[:])
            w2t = cp.tile([P, 3, 67], f32, tag="w2t")
            for c in range(3):
                nc.sync.dma_start(w2t[:, c, :], w2_in[c * P:(c + 1) * P, :])
            b1r = cp.tile([1, 384], f32, tag="b1r")
            nc.sync.dma_start(b1r[:], b1_in[:])
            b1t = cp.tile([P, 384], f32, tag="b1t")
            nc.gpsimd.partition_broadcast(b1t[:], b1r[:])
            b2r = cp.tile([1, 64], f32, tag="b2r")
            nc.sync.dma_start(b2r[:], b2_in[:])
            b2t = cp.tile([P, 64], f32, tag="b2t")
            nc.gpsimd.partition_broadcast(b2t[:], b2r[:])
            idxt = cp.tile([128, 8 * totch], i16, tag="idxt")
            nc.sync.dma_start(idxt[:], idx_in[:])
            dlt = cp.tile([128, totch], f32, tag="dlt")
            nc.sync.dma_start(dlt[:], dl_in[:])
            blt = cp.tile([128, NWIN], f32, tag="blt")
            nc.sync.dma_start(blt[:], bl_in[:])
            stash2 = cp.tile([P, 2 * NWIN], bf16, tag="stash2")

            # ---------------- phase L: h1aug table (replicated) ----------
            for t in range(NT1):
                r0 = t * P
                nr = min(P, N - r0)
                xt = wp.tile([P, P], f32, tag="xt")
                nc.sync.dma_start(xt[:, :nr], xT_in[:, r0:r0 + nr])
                xh = wp.tile([P, P], bf16, tag="xh")
                nc.vector.tensor_copy(xh[:], xt[:])
                xl = wp.tile([P, P], bf16, tag="xl")
                nc.vector.tensor_tensor(out=xl[:], in0=xt[:], in1=xh[:],
                                        op=ALU.subtract)
                h_ps = pacc.tile([P, C1_COLS], f32, tag="acc", name="h_ps")
                nc.tensor.matmul(h_ps[:], lhsT=xh[:], rhs=w1h[:],
                                 start=True, stop=False)
                nc.tensor.matmul(h_ps[:], lhsT=xh[:], rhs=w1l[:],
                                 start=False, stop=False)
                nc.tensor.matmul(h_ps[:], lhsT=xl[:], rhs=w1h[:],
                                 start=False, stop=True)
                ht = wp.tile([P, C1_COLS], f32, tag="ht")
                nc.scalar.copy(ht[:], h_ps[:])
                ones_view = ht[:, 0:396].rearrange("p (h t) -> p h t", t=33)[:, :, 32]
                nc.vector.memset(ones_view, 1.0)
                nc.sync.dma_start(tab1[r0:r0 + nr, :], ht[:nr])

            # ---------------- phase A1: conv1 aggregation ----------------
            def onehot_group(c0, gn, P4tag, Pttag):
                """P4 [P, gn, P] one-hot (bf16) and its transpose Pt (bf16)."""
                P4 = wp.tile([P, GROUP_C, P], bf16, tag=P4tag, name="P4")
                nc.vector.tensor_tensor(
                    out=P4[:, :gn, :],
                    in0=iota[:].unsqueeze(1).to_broadcast([P, gn, P]),
                    in1=dlt[:, c0:c0 + gn].unsqueeze(2).to_broadcast([P, gn, P]),
                    op=ALU.is_equal,
                )
                Pt_ps = ppt.tile([P, GROUP_C, P], bf16, tag="Ptp", name="Pt_ps")
                for c in range(gn):
                    nc.tensor.transpose(Pt_ps[:, c], P4[:, c, :], identb[:])
                Pt = wp.tile([P, GROUP_C, P], bf16, tag=Pttag, name="Pt")
                nc.vector.tensor_copy(Pt[:, :gn], Pt_ps[:, :gn])
                return P4, Pt

            c0 = 0
            for w in range(NWIN):
                s = sched[w]
                nd = s["nd"]
                w0 = w * P
                # a_dst for this window's dst rows (fp32, tiny)
                xw = wp.tile([P, P], f32, tag="xw")
                nc.sync.dma_start(xw[:, :nd], xshT_in[:, w0:w0 + nd])
                ad_ps = psm.tile([P, H1], f32, tag="wps", name="ad_ps")
                nc.tensor.matmul(ad_ps[:], lhsT=xw[:], rhs=w1d[:],
                                 start=True, stop=True)
                # bf16 hi/lo split of a_dst window values
                adwc = wp.tile([P, 2 * H1], bf16, tag="adwc")
                nc.vector.tensor_copy(adwc[:, 0:H1], ad_ps[:])
                nc.vector.tensor_tensor(out=adwc[:, H1:2 * H1], in0=ad_ps[:],
                                        in1=adwc[:, 0:H1], op=ALU.subtract)

                ps_full = pacc.tile([P, C1_COLS], f32, tag="acc", name="ps_full")
                ps_out = ps_full[:, 0:396]
                nch_w = s["chunksA"] + s["chunksB"]
                ci = 0
                for half, nch in (("A", s["chunksA"]), ("B", s["chunksB"])):
                    if nch == 0:
                        continue
                    tab_ap = tab1[0:HALF, :] if half == "A" else tab1[HALF:N, :]
                    gc0 = c0 + ci
                    V = gp.tile([P, CH1, C1_COLS], f32, tag="V1", name="V")
                    for q0 in range(0, nch, GCALL):
                        qn = min(GCALL, nch - q0)
                        nc.gpsimd.dma_gather(
                            out_ap=V[:, q0:q0 + qn, :], in_ap=tab_ap,
                            idxs_ap=idxt[:, 8 * (gc0 + q0): 8 * (gc0 + q0 + qn)],
                            num_idxs=qn * P, num_idxs_reg=qn * P,
                            elem_size=C1_COLS,
                        )
                    for g0 in range(0, nch, GROUP_C):
                        gn = min(GROUP_C, nch - g0)
                        gc = gc0 + g0
                        P4, Pt = onehot_group(gc, gn, "P41", "Pt1")
                        adst_ps = pad.tile([P, GROUP_C, 2 * H1], f32,
                                           tag="adp", name="adst_ps")
                        for c in range(gn):
                            nc.tensor.matmul(adst_ps[:, c], lhsT=Pt[:, c, :],
                                             rhs=adwc[:], start=True, stop=True)
                        wv = wp.tile([P, GROUP_C, H1], f32, tag="wv1")
                        nc.vector.tensor_tensor(
                            out=wv[:, :gn], in0=V[:, g0:g0 + gn, 396:408],
                            in1=adst_ps[:, :gn, 0:H1], op=ALU.add)
                        nc.vector.tensor_tensor(
                            out=wv[:, :gn], in0=wv[:, :gn],
                            in1=adst_ps[:, :gn, H1:2 * H1], op=ALU.add)
                        wl = wp.tile([P, GROUP_C, H1], f32, tag="wl1")
                        nc.vector.tensor_scalar_mul(wl[:, :gn], wv[:, :gn], NEG)
                        nc.vector.tensor_tensor(out=wl[:, :gn], in0=wl[:, :gn],
                                                in1=wv[:, :gn], op=ALU.max)
                        nc.scalar.activation(wv[:, :gn], wl[:, :gn], ACTF.Exp)
                        nc.vector.tensor_tensor(
                            out=V[:, g0:g0 + gn, 0:396].rearrange(
                                "p c (h t) -> p c h t", t=33),
                            in0=V[:, g0:g0 + gn, 0:396].rearrange(
                                "p c (h t) -> p c h t", t=33),
                            in1=wv[:, :gn].unsqueeze(3).to_broadcast(
                                [P, gn, H1, 33]),
                            op=ALU.mult,
                        )
                        Vhi = wp.tile([P, GROUP_C, 396], bf16, tag="Vhi1")
                        nc.scalar.copy(Vhi[:, :gn], V[:, g0:g0 + gn, 0:396])
                        Vlo = wp.tile([P, GROUP_C, 396], bf16, tag="Vlo1")
                        nc.vector.tensor_tensor(
                            out=Vlo[:, :gn], in0=V[:, g0:g0 + gn, 0:396],
                            in1=Vhi[:, :gn], op=ALU.subtract)
                        for c in range(gn):
                            nc.tensor.matmul(
                                ps_out[:], lhsT=P4[:, c, :], rhs=Vhi[:, c],
                                start=(ci + c == 0), stop=False,
                            )
                            nc.tensor.matmul(
                                ps_out[:], lhsT=P4[:, c, :], rhs=Vlo[:, c],
                                start=False, stop=(ci + c == nch_w - 1),
                            )
                        ci += gn
                c0 += nch_w

                # epilogue: normalize, relu(+bias), conv2 rows
                rec = wp.tile([P, H1], f32, tag="rec")
                nc.vector.tensor_scalar_max(
                    rec[:], ps_out[:].rearrange(
                        "p (h t) -> p h t", t=33)[:, :, 32], 1e-30)
                nc.vector.reciprocal(rec[:], rec[:])
                out1 = wp.tile([P, 384], f32, tag="out1")
                nc.vector.tensor_tensor(
                    out=out1[:].rearrange("p (h t) -> p h t", t=32),
                    in0=ps_out[:].rearrange(
                        "p (h t) -> p h t", t=33)[:, :, 0:32],
                    in1=rec[:].unsqueeze(2).to_broadcast([P, H1, 32]),
                    op=ALU.mult,
                )
                nc.vector.tensor_tensor(out=out1[:], in0=out1[:], in1=b1t[:],
                                        op=ALU.add)
                nc.vector.tensor_scalar_max(out1[:], out1[:], 0.0)
                o1T_ps = psm.tile([P, 3, P], f32, tag="wps", name="o1T_ps")
                for c in range(3):
                    nc.tensor.transpose(o1T_ps[:, c],
                                        out1[:, c * P:(c + 1) * P], ident[:])
                o1T = wp.tile([P, 3, P], f32, tag="o1T")
                nc.vector.tensor_copy(o1T[:], o1T_ps[:])
                h2_ps = psm.tile([P, 67], f32, tag="wps", name="h2_ps")
                for c in range(3):
                    nc.tensor.matmul(h2_ps[:], lhsT=o1T[:, c, :],
                                     rhs=w2t[:, c, :],
                                     start=(c == 0), stop=(c == 2))
                nc.vector.tensor_copy(stash2[:, 2 * w:2 * w + 1], h2_ps[:, 66:67])
                nc.vector.tensor_tensor(out=stash2[:, 2 * w + 1:2 * w + 2],
                                        in0=h2_ps[:, 66:67],
                                        in1=stash2[:, 2 * w:2 * w + 1],
                                        op=ALU.subtract)
                h2t = wp.tile([P, C2_COLS], f32, tag="h2t")
                nc.scalar.copy(h2t[:, 0:67], h2_ps[:])
                nc.vector.memset(h2t[:, 64:65], 1.0)
                nc.vector.memset(h2t[:, 67:], 0.0)
                nc.sync.dma_start(shard2[w0:w0 + nd, :], h2t[:nd])

            # ---------------- allgather conv2 table ----------------------
            nc.gpsimd.collective_compute(
                "AllGather", mybir.AluOpType.bypass,
                replica_groups=[list(range(NC))],
                ins=[shard2[:].opt()],
                outs=[tab2[:].opt()],
            )

            # ---------------- phase A2: conv2 aggregation + pooling ------
            pool_ps = ppool.tile([GC, 64], f32, tag="pool_ps")
            c0 = 0
            for w in range(NWIN):
                s = sched[w]
                nd = s["nd"]
                ps2_full = pacc.tile([P, C1_COLS], f32, tag="acc", name="ps2_full")
                ps2 = ps2_full[:, 0:65]
                nch_w = s["chunksA"] + s["chunksB"]
                ci = 0
                for half, nch in (("A", s["chunksA"]), ("B", s["chunksB"])):
                    if nch == 0:
                        continue
                    tab_ap = tab2[0:HALF, :] if half == "A" else tab2[HALF:N, :]
                    gc0 = c0 + ci
                    V2 = gp.tile([P, CH1, C2_COLS], f32, tag="V2", name="V2")
                    for q0 in range(0, nch, GCALL):
                        qn = min(GCALL, nch - q0)
                        nc.gpsimd.dma_gather(
                            out_ap=V2[:, q0:q0 + qn, :], in_ap=tab_ap,
                            idxs_ap=idxt[:, 8 * (gc0 + q0): 8 * (gc0 + q0 + qn)],
                            num_idxs=qn * P, num_idxs_reg=qn * P,
                            elem_size=C2_COLS,
                        )
                    for g0 in range(0, nch, GROUP_C):
                        gn = min(GROUP_C, nch - g0)
                        gc = gc0 + g0
                        P4, Pt = onehot_group(gc, gn, "P41", "Pt1")
                        adst_ps = pad.tile([P, GROUP_C, 2 * H1], f32,
                                           tag="adp", name="adst2_ps")
                        for c in range(gn):
                            nc.tensor.matmul(adst_ps[:, c, 0:2],
                                             lhsT=Pt[:, c, :],
                                             rhs=stash2[:, 2 * w:2 * w + 2],
                                             start=True, stop=True)
                        wv = wp.tile([P, GROUP_C], f32, tag="wv2")
                        nc.vector.tensor_tensor(
                            out=wv[:, :gn], in0=V2[:, g0:g0 + gn, 65],
                            in1=adst_ps[:, :gn, 0], op=ALU.add)
                        nc.vector.tensor_tensor(
                            out=wv[:, :gn], in0=wv[:, :gn],
                            in1=adst_ps[:, :gn, 1], op=ALU.add)
                        wl = wp.tile([P, GROUP_C], f32, tag="wl2")
                        nc.vector.tensor_scalar_mul(wl[:, :gn], wv[:, :gn], NEG)
                        nc.vector.tensor_tensor(out=wl[:, :gn], in0=wl[:, :gn],
                                                in1=wv[:, :gn], op=ALU.max)
                        nc.scalar.activation(wv[:, :gn], wl[:, :gn], ACTF.Exp)
                        nc.vector.tensor_tensor(
                            out=V2[:, g0:g0 + gn, 0:65],
                            in0=V2[:, g0:g0 + gn, 0:65],
                            in1=wv[:, :gn].unsqueeze(2).to_broadcast(
                                [P, gn, 65]),
                            op=ALU.mult,
                        )
                        V2hi = wp.tile([P, GROUP_C, 65], bf16, tag="V2hi")
                        nc.scalar.copy(V2hi[:, :gn], V2[:, g0:g0 + gn, 0:65])
                        V2lo = wp.tile([P, GROUP_C, 65], bf16, tag="V2lo")
                        nc.vector.tensor_tensor(
                            out=V2lo[:, :gn], in0=V2[:, g0:g0 + gn, 0:65],
                            in1=V2hi[:, :gn], op=ALU.subtract)
                        for c in range(gn):
                            nc.tensor.matmul(
                                ps2[:], lhsT=P4[:, c, :], rhs=V2hi[:, c],
                                start=(ci + c == 0), stop=False,
                            )
                            nc.tensor.matmul(
                                ps2[:], lhsT=P4[:, c, :], rhs=V2lo[:, c],
                                start=False, stop=(ci + c == nch_w - 1),
                            )
                        ci += gn
                c0 += nch_w

                rec2 = wp.tile([P, 1], f32, tag="rec2")
                nc.vector.tensor_scalar_max(rec2[:], ps2[:, 64:65], 1e-30)
                nc.vector.reciprocal(rec2[:], rec2[:])
                out2 = wp.tile([P, 64], f32, tag="out2")
                nc.vector.tensor_scalar(out=out2[:], in0=ps2[:, 0:64],
                                        scalar1=rec2[:, 0:1], scalar2=None,
                                        op0=ALU.mult)
                nc.vector.tensor_tensor(out=out2[:], in0=out2[:], in1=b2t[:],
                                        op=ALU.add)
                Pg = wp.tile([P, GC], f32, tag="Pg")
                nc.vector.tensor_tensor(
                    out=Pg[:], in0=iota[:, 0:GC],
                    in1=blt[:, w:w + 1].to_broadcast([P, GC]),
                    op=ALU.is_equal)
                nc.tensor.matmul(pool_ps[:], lhsT=Pg[:nd, :], rhs=out2[:nd, :],
                                 start=(w == 0), stop=(w == NWIN - 1))

            pool_sb = cp.tile([GC, 64], f32, tag="pool_sb")
            nc.vector.tensor_copy(pool_sb[:], pool_ps[:])
            nc.sync.dma_start(pool_out[:], pool_sb[:])

    nc.compile()
    return nc


# ------------------------------------------------------------------ driver

_CACHE = {}


def _run(inputs, trace=False):
    x = np.asarray(inputs["x"], np.float32)
    xT = np.ascontiguousarray(x.T)
    ed = _build_edges(inputs["edge_index"], inputs["batch"])
    W1aug, W2aug = _build_weights(
        inputs["W1"], inputs["att_src1"], inputs["att_dst1"],
        inputs["W2"], inputs["att_src2"], inputs["att_dst2"])
    w1h, w1l = _bf_split(W1aug)
    b1 = np.asarray(inputs["bias1"], np.float32).reshape(1, 384)
    b2 = np.asarray(inputs["bias2"], np.float32).reshape(1, 64)

    key = (ed["totch"], ed["GC"],
           tuple((s["nd"], s["chunksA"], s["chunksB"]) for s in ed["sched"]))
    if key not in _CACHE:
        _CACHE.clear()
        _CACHE[key] = _build_program(ed["sched"], ed["totch"], ed["GC"])
    nc = _CACHE[key]

    in_maps = []
    for k in range(NC):
        in_maps.append(dict(
            xT=xT,
            xshT=np.ascontiguousarray(xT[:, k * SH:(k + 1) * SH]),
            w1h=w1h, w1l=w1l,
            w1d=np.ascontiguousarray(W1aug[:, 408:420]),
            w2aug=np.ascontiguousarray(W2aug),
            b1=b1, b2=b2,
            idx16=np.ascontiguousarray(ed["idx16"][k]),
            dstloc=np.ascontiguousarray(ed["dstloc"][k]),
            batchloc=np.ascontiguousarray(ed["batchloc"][k]),
        ))
    res = run_bass_kernel_spmd(nc, in_maps, core_ids=list(range(NC)),
                               trace=trace)

    sums = np.zeros((G, 64), np.float64)
    GCn = ed["GC"]
    for k in range(NC):
        lo = int(ed["g_lo"][k])
        hi = min(lo + GCn, G)
        sums[lo:hi] += res.results[k]["pool_out"][:hi - lo]
    cnts = np.bincount(np.asarray(inputs["batch"], np.int64),
                       minlength=G).astype(np.float64)
    out = (sums / np.maximum(cnts, 1.0)[:, None]).astype(np.float32)
    return out, res


def kernel(**inputs) -> np.ndarray:
    out, _ = _run(inputs, trace=False)
    return out
